# revision 65
# baseline (speedup 1.0000x reference)
"""Multi-head self-attention with RoPE, sharded over 8 TRN2 NeuronCores.

Sharding: tensor-parallel over heads (2 heads/core) for QKV projections and
attention; one AllToAll redistributes normalized attention outputs from
head-sharded to sequence-sharded so each core computes 1/8 of the output
projection columns.

v2 design notes (vs the v1 baseline):
- V is projected directly in transposed orientation (keys on partitions) by
  swapping matmul operand roles, removing the PE transposes + DVE extracts.
- RoPE uses partition-offset tensor_muls straight out of qT/kT (no swap-copy
  staging), in bf16 for DVE 2x throughput; cos/sin tables ship as bf16.
- Causal masking runs on the idle GPSIMD(Pool) engine with a single [128,128]
  triangle tile; score/exp/PV work on diagonal blocks is column-restricted to
  the valid region.
- Attention outputs are normalized on the producing core (reciprocal +
  broadcast-matmul), so the AllToAll payload is a single bf16 [8,128,512]
  buffer: half the bytes and one collective instead of two zero-padded ones.
- Out-projection weights are bf16 and fully SBUF-resident before phase 3.

Hardcoded problem shape: B=1, S=4096, D=1024, H=16, hd=64, theta=10000.
"""

import math

import numpy as np

import concourse.bass as bass
import concourse.mybir as mybir
import concourse.tile as tile
from concourse import bacc
from concourse.bass_utils import run_bass_kernel_spmd

N_CORES = 8
D_MODEL = 1024
NUM_HEADS = 16
HEAD_DIM = 64
THETA = 10000.0
P = 128  # partitions; also = 2 heads x 64 dims per core
KD = D_MODEL // 128  # 8 contraction tiles for the projections

F32 = mybir.dt.float32
F32R = mybir.dt.float32r
BF16 = mybir.dt.bfloat16
FP8 = mybir.dt.float8e4
EXP = mybir.ActivationFunctionType.Exp
DR = mybir.MatmulPerfMode.DoubleRow


def build(seq: int, p12_reps: int = 1, p3_reps: int = 1, parts: str = "full"):
    """Build the SPMD Bass program for sequence length `seq`.

    p12_reps > 1 wraps phases 1+2 (projections + attention) in an on-device
    For_i loop; p3_reps > 1 unrolls phase 3 (A2A + out-proj) — both exist
    for wall-clock timing above the axon dispatch floor."""
    CH = min(512, seq)          # free-dim chunk for matmuls / PSUM banks
    NCH = seq // CH             # number of seq chunks
    KB = seq // 128             # key blocks
    KBC = CH // 128             # key blocks per chunk (4 at CH=512)
    SW = seq // N_CORES         # per-core output seq shard (== CH here)

    nc = bacc.Bacc("TRN2", num_devices=N_CORES)

    xt = nc.dram_tensor("xt", [D_MODEL, seq], BF16, kind="ExternalInput")
    wq = nc.dram_tensor("wq", [P, D_MODEL], BF16, kind="ExternalInput")
    wk = nc.dram_tensor("wk", [P, D_MODEL], BF16, kind="ExternalInput")
    wvt = nc.dram_tensor("wvt", [P, D_MODEL], BF16, kind="ExternalInput")
    wo = nc.dram_tensor("wo", [P, KD * D_MODEL], BF16, kind="ExternalInput")
    ctab = nc.dram_tensor("ctab", [P, seq], BF16, kind="ExternalInput")
    stab = nc.dram_tensor("stab", [P, seq], BF16, kind="ExternalInput")
    trid = nc.dram_tensor("tri", [P, 128], BF16, kind="ExternalInput")
    e0d = nc.dram_tensor("e0", [1, P], F32R, kind="ExternalInput")
    e1d = nc.dram_tensor("e1", [1, P], F32R, kind="ExternalInput")
    out_d = nc.dram_tensor("out", [D_MODEL, SW], BF16, kind="ExternalOutput")

    with tile.TileContext(nc) as tc:
        with (
            tc.tile_pool(name="const", bufs=1) as cpool,
            tc.tile_pool(name="mats", bufs=1) as mpool,
            tc.tile_pool(name="xt", bufs=2) as xpool,
            tc.tile_pool(name="sc", bufs=2) as spool,
            tc.tile_pool(name="pt", bufs=6) as ptpool,
            tc.tile_pool(name="at", bufs=1) as apool,
            tc.tile_pool(name="ps", bufs=2, space="PSUM") as pspool,
            tc.tile_pool(name="pss", bufs=2, space="PSUM") as psspool,
            tc.tile_pool(name="dram", bufs=1, space="DRAM") as dpool,
        ):
            # ---- constants ----
            w_sb = {}
            for name, wsrc in (("q", wq), ("k", wk), ("vt", wvt)):
                t = cpool.tile([P, D_MODEL], BF16, tag=f"w{name}")
                # ACT ring: keeps the SP ring free for chunk 0's xt tiles
                nc.scalar.dma_start(out=t[:], in_=wsrc[:])
                w_sb[name] = t
            # ct/st/wo are loaded lazily (per-chunk / after chunk 0) so the
            # startup queues belong to chunk 0's xt tiles
            ct = cpool.tile([P, seq], BF16, tag="ct")
            st = cpool.tile([P, seq], BF16, tag="st")
            tri = cpool.tile([P, 128], BF16, tag="tri")
            nc.scalar.dma_start(out=tri[:], in_=trid[:])
            e0 = cpool.tile([1, P], F32R, tag="e0")
            nc.scalar.dma_start(out=e0[:], in_=e0d[:])
            e1 = cpool.tile([1, P], F32R, tag="e1")
            nc.scalar.dma_start(out=e1[:], in_=e1d[:])
            wo_sb = cpool.tile([P, KD * D_MODEL], BF16, tag="wo")

            # ---- persistent matrices ----
            qT = mpool.tile([P, seq], BF16, tag="qT")  # rows: 2 heads x 64
            kT = mpool.tile([P, seq], BF16, tag="kT")
            vnat = mpool.tile([P, KB * 130], BF16, tag="vnat")
            # ones columns (cols 64 and 129 of each 130-wide block) feed the
            # softmax denominator row of the PV matmul
            vv = vnat[:].rearrange("p (k c) -> p k c", c=130)
            nc.gpsimd.memset(vv[:, :, 64:65], 1.0)
            nc.gpsimd.memset(vv[:, :, 129:130], 1.0)

            a2a_in = dpool.tile([N_CORES, P, SW], BF16, tag="a2a_in")
            a2a_out = dpool.tile([N_CORES, P, SW], BF16, tag="a2a_out")

            # ---- warmup: preload the Exp activation table while the
            # first DMAs are in flight (real hw pays ~1.3us on first use) ----
            wz = cpool.tile([1, 128], BF16, tag="warm")
            nc.gpsimd.memset(wz[:], 0.0)
            wpt = cpool.tile([1, 128], BF16, tag="warmo")
            nc.scalar.activation(wpt[:], wz[:], EXP)

            def proj_stages(sc):
                """Q/K/V projection + rope for chunk sc as a list of stage
                closures, so the PE work can be dribbled into the previous
                chunk's attention stream (one stage per few key blocks)
                instead of landing as one ~5us block that starves ACT."""
                sl = bass.ts(sc, CH)
                xts = []

                def st_dma():
                    for k in range(KD):
                        t = xpool.tile([P, CH], BF16, tag=f"xt{k}",
                                       name=f"xt_{sc}_{k}")
                        nc.sync.dma_start(
                            out=t[:], in_=xt[128 * k:128 * (k + 1), sl]
                        )
                        xts.append(t)
                    nc.sync.dma_start(out=ct[:, sl], in_=ctab[:, sl])
                    nc.sync.dma_start(out=st[:, sl], in_=stab[:, sl])

                def st_qk(name, dst):
                    ps = pspool.tile([P, CH], F32, tag="mm",
                                     name=f"proj_{sc}_{name}")
                    for k in range(KD):
                        nc.tensor.matmul(
                            ps[:],
                            w_sb[name][:, bass.ts(k, 128)],
                            xts[k][:],
                            start=(k == 0),
                            stop=(k == KD - 1),
                        )
                    nc.vector.tensor_copy(dst[:, sl], ps[:])

                def st_rope():
                    # rope in-place on qT/kT (bf16, DVE 2x): copy the
                    # half-swapped rows (copies may cross partition
                    # offsets; tensor-tensor ops may not), then aligned ops
                    for mi, mat in ((0, qT), (1, kT)):
                        tm = spool.tile([P, CH], BF16, tag="tm",
                                        name=f"tm_{sc}_{mi}")
                        for h in (0, 1):
                            for half in (0, 1):
                                d0 = 64 * h + 32 * half
                                s0 = 64 * h + 32 * (1 - half)
                                nc.vector.tensor_copy(
                                    tm[d0:d0 + 32, :], mat[s0:s0 + 32, sl]
                                )
                        nc.vector.tensor_mul(tm[:], tm[:], st[:, sl])
                        nc.vector.tensor_mul(mat[:, sl], mat[:, sl],
                                             ct[:, sl])
                        nc.vector.tensor_add(mat[:, sl], mat[:, sl], tm[:])

                def st_v(r):
                    # V directly transposed: [128 keys, 128 dims] region
                    # (shares the "mm" tag; only cols 0:128 are used)
                    psv = pspool.tile([P, CH], F32, tag="mm",
                                      name=f"vtr_{sc}_{r}")
                    rs = bass.ts(r, 128)
                    for j in range(KD):
                        nc.tensor.matmul(
                            psv[:, 0:128],
                            xts[j][:, rs],
                            w_sb["vt"][:, bass.ts(j, 128)],
                            start=(j == 0),
                            stop=(j == KD - 1),
                        )
                    kb = sc * KBC + r
                    nc.vector.tensor_copy(
                        vnat[:, 130 * kb:130 * kb + 64], psv[:, 0:64]
                    )
                    nc.vector.tensor_copy(
                        vnat[:, 130 * kb + 65:130 * kb + 129],
                        psv[:, 64:128],
                    )

                stages = [st_dma,
                          lambda: st_qk("q", qT),
                          lambda: st_qk("k", kT),
                          st_rope]
                stages += [(lambda r=r: st_v(r)) for r in range(KBC)]
                return stages

            def emit_proj_chunk(sc):
                for f in proj_stages(sc):
                    f()

            def emit_attn_chunk(qc, inject=None, epi=None):
                """Attention for query chunk qc (needs proj chunks 0..qc).

                Per key block: S^T for both heads lands in one [128, 2*CH]
                PSUM tile ([0:CH]=h0, [CH:2CH]=h1). Diagonal-band blocks run
                FIRST; their score/exp/PV work is restricted to the valid
                column range [128j, CH), and the triangle sub-block is masked
                on GPSIMD.

                `inject` is called after the diagonal group: the next chunk's
                projection matmuls slot into the PE stream there, overlapping
                the Activation engine's exp backlog so ACT never starves at
                chunk boundaries.

                `epi` is the PREVIOUS chunk's deferred epilogue; it is
                emitted just before this chunk's first PV so the reciprocal
                chain overlaps this chunk's diagonal scores instead of
                stalling the PE at the chunk boundary. Returns this chunk's
                epilogue closure for the same treatment."""
                kbmax = (qc + 1) * KBC
                psu = {}
                for h in (0, 1):
                    psu[h] = pspool.tile([65, CH], F32, tag="u",
                                         name=f"psu_{qc}_{h}")
                kb_order = (list(range(kbmax - KBC, kbmax))
                            + list(range(0, kbmax - KBC)))

                def emit_pv(kb, ki, pt, q0):
                    for h in (0, 1):
                        nc.tensor.matmul(
                            psu[h][:, q0:CH],
                            vnat[:, 130 * kb + 65 * h:
                                 130 * kb + 65 * (h + 1)],
                            pt[:, CH * h + q0:CH * (h + 1)],
                            start=(ki == 0),
                            stop=(ki == kbmax - 1),
                        )

                DEFER = 3       # PV trails exp by this many key blocks
                pending = []    # (kb, ki, pt, q0) with PV deferred
                inject = list(inject) if inject else []
                for ki, kb in enumerate(kb_order):
                    if ki == 1 and epi is not None:
                        epi()
                        epi = None
                    # dribble the next chunk's projection stages into the
                    # PE stream: DMAs right away, one compute stage per two
                    # key blocks after the diagonal group
                    if inject and (ki == 1 or
                                   (ki >= KBC and (ki - KBC) % 2 == 0)):
                        inject.pop(0)()
                    j = kb - (kbmax - KBC)  # diag index if >= 0
                    lastkb = (ki == kbmax - 1)
                    # column-restrict diagonal blocks except the last kb of
                    # the chunk (its stop=True must cover the full psu width)
                    q0 = 128 * j if (j >= 1 and not lastkb) else 0
                    pss = psspool.tile([P, 2 * CH], F32, tag="s",
                                       name=f"sc_{qc}_{kb}")
                    for h in (0, 1):
                        nc.tensor.matmul(
                            pss[:, CH * h + q0:CH * (h + 1)],
                            kT[64 * h:64 * (h + 1), bass.ts(kb, 128)],
                            qT[64 * h:64 * (h + 1),
                               qc * CH + q0:(qc + 1) * CH],
                            start=True,
                            stop=True,
                        )
                    pt = ptpool.tile([P, 2 * CH], BF16, tag="pt",
                                     name=f"pt_{qc}_{kb}")
                    if q0:
                        for h in (0, 1):
                            nc.scalar.activation(
                                pt[:, CH * h + q0:CH * (h + 1)],
                                pss[:, CH * h + q0:CH * (h + 1)],
                                EXP,
                            )
                    else:
                        nc.scalar.activation(pt[:], pss[:], EXP)
                    if j >= 0:
                        if lastkb and j >= 1:
                            # full-width last kb on a diagonal block (qc==0):
                            # zero the columns left of the band
                            for h in (0, 1):
                                nc.gpsimd.memset(
                                    pt[:, CH * h:CH * h + 128 * j], 0.0
                                )
                        for h in (0, 1):
                            o = CH * h + 128 * j
                            nc.gpsimd.tensor_mul(
                                pt[:, o:o + 128], pt[:, o:o + 128], tri[:]
                            )
                    if parts == "attn_s":
                        continue
                    pending.append((kb, ki, pt, q0))
                    if len(pending) > DEFER:
                        emit_pv(*pending.pop(0))
                while pending and parts != "attn_s":
                    emit_pv(*pending.pop(0))
                for f in inject:
                    f()
                if parts in ("attn_s", "attn_pv"):
                    return None

                def epilogue(final=False):
                    # normalize U by the denominator row and stage the bf16
                    # payload for the AllToAll; the final chunk's chain gates
                    # the AllToAll, so split its work across DVE and ACT
                    rcp = []
                    with nc.allow_low_precision(
                            reason="f32r tag only; PE rounds on read"):
                        for h in (0, 1):
                            r = spool.tile([1, CH], F32R, tag=f"rs{h}",
                                           name=f"rs_{qc}_{h}")
                            nc.vector.reciprocal(r[:], psu[h][64:65, :])
                            rcp.append(r)
                    rb = pspool.tile([P, CH], F32, tag="mm", name=f"rb_{qc}")
                    nc.tensor.matmul(rb[:], e0[:], rcp[0][:],
                                     start=True, stop=False)
                    nc.tensor.matmul(rb[:], e1[:], rcp[1][:],
                                     start=False, stop=True)
                    # copy U halves into stg (copies may remap partitions),
                    # then normalize in place with partition-aligned muls
                    stg = spool.tile([P, CH], BF16, tag="stg",
                                     name=f"stg_{qc}")
                    for h in (0, 1):
                        hs = slice(64 * h, 64 * (h + 1))
                        nc.vector.tensor_copy(stg[hs, :], psu[h][0:64, :])
                        nc.vector.tensor_mul(stg[hs, :], stg[hs, :],
                                             rb[hs, :])
                    for half in (0, 1):
                        hs = bass.ts(half, CH // 2)
                        nc.sync.dma_start(out=a2a_in[qc, :, hs],
                                          in_=stg[:, hs])

                return epilogue

            def emit_p12():
                if parts.split("_")[0] in ("dma", "proj", "rope", "noil"):
                    for sc in range(NCH):
                        emit_proj_chunk(sc)
                    if parts == "noil":
                        for sc in range(NCH):
                            emit_attn_chunk(sc)
                    return
                emit_proj_chunk(0)
                for e in range(KD):
                    nc.sync.dma_start(
                        out=wo_sb[:, bass.ts(e, D_MODEL)],
                        in_=wo[:, bass.ts(e, D_MODEL)],
                    )
                DEFER_EPI = True
                epi = None
                for qc in range(NCH):
                    nxt = qc + 1
                    ret = emit_attn_chunk(
                        qc,
                        inject=proj_stages(nxt) if nxt < NCH else None,
                        epi=epi,
                    )
                    if DEFER_EPI:
                        epi = ret
                    elif ret is not None:
                        ret()
                if epi is not None:
                    epi(final=True)

            def emit_cc():
                nc.gpsimd.collective_compute(
                    "AllToAll",
                    mybir.AluOpType.bypass,
                    replica_groups=[list(range(N_CORES))],
                    ins=[a2a_in.opt()],
                    outs=[a2a_out.opt()],
                )

            def emit_p3(cc_done):
                if not cc_done:
                    emit_cc()
                ats = []
                for i in range(N_CORES):
                    at = apool.tile([P, SW], BF16, tag=f"at{i}",
                                    name=f"at_{i}")
                    eng = nc.sync if i % 2 == 0 else nc.scalar
                    eng.dma_start(out=at[:], in_=a2a_out[i])
                    ats.append(at)
                for e in range(KD):
                    pso = pspool.tile([P, SW], F32, tag="mm",
                                      name=f"pso_{e}")
                    for i in range(N_CORES):
                        nc.tensor.matmul(
                            pso[:],
                            wo_sb[:, D_MODEL * e + 128 * i:
                                  D_MODEL * e + 128 * (i + 1)],
                            ats[i][:],
                            start=(i == 0),
                            stop=(i == N_CORES - 1),
                        )
                    ot = ptpool.tile([P, SW], BF16, tag="ot", name=f"ot_{e}")
                    # alternate the PSUM drain between DVE and ACT (both
                    # idle in phase 3) and spread out DMAs over two rings
                    if e % 2 == 0:
                        nc.vector.tensor_copy(ot[:], pso[:])
                    else:
                        nc.scalar.copy(ot[:], pso[:])
                    eng = nc.sync if e % 2 == 0 else nc.scalar
                    eng.dma_start(out=out_d[bass.ts(e, 128)], in_=ot[:])

            if p12_reps == 1:
                emit_p12()
                if parts == "full":
                    emit_cc()
            else:
                with tc.For_i(0, p12_reps, 1):
                    emit_p12()
            if parts == "full":
                for r3 in range(p3_reps):
                    emit_p3(cc_done=(p12_reps == 1 and r3 == 0))

    nc.finalize()
    return nc


def prepare_in_maps(in_features, token_positions, Wq, Wk, Wv, Wo, seq):
    """Host-side staging: shard/transform full inputs into per-core maps."""
    import ml_dtypes
    bf16 = ml_dtypes.bfloat16
    x = np.ascontiguousarray(np.asarray(in_features, dtype=np.float32)[0])
    pos = np.asarray(token_positions).reshape(-1)[:seq].astype(np.float64)

    xt = np.ascontiguousarray(x.T).astype(bf16)  # [D, S]

    # RoPE tables in rotate-half form after pair permutation.
    inv_freq = THETA ** (-np.arange(0, HEAD_DIM, 2, dtype=np.float64)
                         / HEAD_DIM)
    ang = pos[:, None] * inv_freq[None, :]  # [S, 32]
    cos = np.cos(ang).T.astype(np.float32)  # [32, S]
    sin = np.sin(ang).T.astype(np.float32)
    ctab = np.ascontiguousarray(np.tile(cos, (4, 1))).astype(bf16)
    stab = np.ascontiguousarray(
        np.concatenate([-sin, sin, -sin, sin], axis=0)
    ).astype(bf16)

    perm = np.concatenate(
        [np.arange(0, HEAD_DIM, 2), np.arange(1, HEAD_DIM, 2)]
    )  # within-head: evens then odds

    tri = np.triu(np.ones((128, 128), dtype=np.float32)).astype(bf16)
    e0_host = np.zeros((1, 128), dtype=np.float32)
    e0_host[0, 0:64] = 1.0
    e1_host = np.zeros((1, 128), dtype=np.float32)
    e1_host[0, 64:128] = 1.0

    import ml_dtypes as _mld
    WoT = np.ascontiguousarray(np.asarray(Wo, dtype=np.float32).T)  # [d, e]
    wo_packed = np.empty((128, KD * D_MODEL), dtype=np.float32)
    for e in range(KD):
        for i in range(KD):
            wo_packed[:, D_MODEL * e + 128 * i: D_MODEL * e + 128 * (i + 1)] \
                = WoT[128 * i:128 * (i + 1), 128 * e:128 * (e + 1)]
    wo_packed = wo_packed.astype(bf16)

    def pack_w(Wc):
        # Wc: [128 out, 1024 in] -> WT [1024, 128] -> [128, 8*128] k-tiled
        WT = np.ascontiguousarray(Wc.T)
        return np.ascontiguousarray(
            WT.reshape(KD, 128, 128).transpose(1, 0, 2).reshape(128, KD * 128)
        ).astype(bf16)

    in_maps = []
    for c in range(N_CORES):
        rows = slice(128 * c, 128 * (c + 1))
        Wq_r = np.asarray(Wq, dtype=np.float32)[rows].reshape(2, 64, D_MODEL)
        Wq_c = (Wq_r[:, perm, :] / math.sqrt(HEAD_DIM)).reshape(128, D_MODEL)
        Wk_r = np.asarray(Wk, dtype=np.float32)[rows].reshape(2, 64, D_MODEL)
        Wk_c = Wk_r[:, perm, :].reshape(128, D_MODEL)
        # wvt block j = WvT[128j:128(j+1), :] = Wv_core[:, 128j:...].T
        WvT = np.ascontiguousarray(
            np.asarray(Wv, dtype=np.float32)[rows].T
        )  # [1024 in, 128 out]
        wvt_packed = np.ascontiguousarray(
            WvT.reshape(KD, 128, 128).transpose(1, 0, 2).reshape(
                128, KD * 128)
        ).astype(bf16)
        in_maps.append({
            "xt": xt,
            "wq": pack_w(Wq_c),
            "wk": pack_w(Wk_c),
            "wvt": wvt_packed,
            "wo": wo_packed,
            "ctab": ctab,
            "stab": stab,
            "tri": tri,
            "e0": e0_host,
            "e1": e1_host,
        })
    return in_maps


_BUILD_CACHE = {}


def _get_nc(seq, p12_reps=1, p3_reps=1, parts="full"):
    key = (seq, p12_reps, p3_reps, parts)
    if key not in _BUILD_CACHE:
        _BUILD_CACHE[key] = build(seq, p12_reps, p3_reps, parts)
    return _BUILD_CACHE[key]


def postprocess(results, seq, in_dtype):
    SW = seq // N_CORES
    out = np.empty((seq, D_MODEL), dtype=np.float32)
    for c in range(N_CORES):
        out[SW * c:SW * (c + 1), :] = results[c]["out"].T
    return out.reshape(1, seq, D_MODEL).astype(in_dtype)


def kernel(in_features, token_positions, Wq, Wk, Wv, Wo):
    in_dtype = np.asarray(in_features).dtype
    B, S, D = np.asarray(in_features).shape
    assert B == 1 and D == D_MODEL

    nc = _get_nc(S)
    in_maps = prepare_in_maps(in_features, token_positions, Wq, Wk, Wv, Wo, S)
    res = run_bass_kernel_spmd(nc, in_maps, list(range(N_CORES)), trace=False)
    return postprocess(res.results, S, in_dtype)


# revision 71
# speedup vs baseline: 1.2272x; 1.2272x over previous
"""Multi-head self-attention with RoPE, sharded over 8 TRN2 NeuronCores.

Sharding: tensor-parallel over heads (2 heads/core) for QKV projections and
attention; one AllToAll redistributes normalized attention outputs from
head-sharded to sequence-sharded so each core computes 1/8 of the output
projection columns.

v2 design notes (vs the v1 baseline, sim 362us -> 262us):
- V is projected directly in transposed orientation (keys on partitions) by
  swapping matmul operand roles, removing the PE transposes + DVE extracts.
- RoPE runs in bf16 on qT/kT in place (DVE 2x mode); cos/sin tables ship as
  bf16 and are DMA'd per chunk so startup isn't gated on 4MB of tables.
- Causal masking runs on the idle GPSIMD(Pool) engine with a single [128,128]
  triangle tile; score/exp/PV work on diagonal blocks is column-restricted to
  the valid region.
- The next chunk's projection is emitted as small stages dribbled between
  attention key blocks (and each chunk's epilogue is deferred into the next
  chunk), so the in-order PE queue keeps the Activation engine's exp stream
  fed at chunk boundaries; PV trails exp by DEFER key blocks for elasticity.
- Attention outputs are normalized on the producing core (reciprocal +
  broadcast-matmul), so the AllToAll payload is a single bf16 [8,128,512]
  buffer: half the bytes and one collective instead of two zero-padded ones.
- Out-projection weights are bf16 and SBUF-resident before phase 3; the
  output is written bf16 and widened on the host.
- fp8 was tried for scores/PV/payload/out-proj and REJECTED: with random-sign
  contractions the ~5% e4m3 element error lands directly on the output
  (rel err 4e-2 > the 2e-2 gate). Exp is Activation-engine-only on TRN2
  (verifier NCC_IBIR606), so the ~145us exp stream is the late-phase floor.

Hardcoded problem shape: B=1, S=4096, D=1024, H=16, hd=64, theta=10000.
"""

import math

import numpy as np

import concourse.bass as bass
import concourse.mybir as mybir
import concourse.tile as tile
from concourse import bacc
from concourse.bass_utils import run_bass_kernel_spmd

N_CORES = 8
D_MODEL = 1024
NUM_HEADS = 16
HEAD_DIM = 64
THETA = 10000.0
P = 128  # partitions; also = 2 heads x 64 dims per core
KD = D_MODEL // 128  # 8 contraction tiles for the projections

F32 = mybir.dt.float32
F32R = mybir.dt.float32r
BF16 = mybir.dt.bfloat16
FP8 = mybir.dt.float8e4
EXP = mybir.ActivationFunctionType.Exp
DR = mybir.MatmulPerfMode.DoubleRow


def build(seq: int, p12_reps: int = 1, p3_reps: int = 1, parts: str = "full"):
    """Build the SPMD Bass program for sequence length `seq`.

    p12_reps > 1 wraps phases 1+2 (projections + attention) in an on-device
    For_i loop; p3_reps > 1 unrolls phase 3 (A2A + out-proj) — both exist
    for wall-clock timing above the axon dispatch floor."""
    CH = min(512, seq)          # free-dim chunk for matmuls / PSUM banks
    NCH = seq // CH             # number of seq chunks
    KB = seq // 128             # key blocks
    KBC = CH // 128             # key blocks per chunk (4 at CH=512)
    SW = seq // N_CORES         # per-core output seq shard (== CH here)

    nc = bacc.Bacc("TRN2", num_devices=N_CORES)

    xt = nc.dram_tensor("xt", [D_MODEL, seq], BF16, kind="ExternalInput")
    wq = nc.dram_tensor("wq", [P, D_MODEL], BF16, kind="ExternalInput")
    wk = nc.dram_tensor("wk", [P, D_MODEL], BF16, kind="ExternalInput")
    wvt = nc.dram_tensor("wvt", [P, D_MODEL], BF16, kind="ExternalInput")
    wo = nc.dram_tensor("wo", [P, KD * D_MODEL], BF16, kind="ExternalInput")
    ctab = nc.dram_tensor("ctab", [P, seq], BF16, kind="ExternalInput")
    stab = nc.dram_tensor("stab", [P, seq], BF16, kind="ExternalInput")
    trid = nc.dram_tensor("tri", [P, 128], BF16, kind="ExternalInput")
    e0d = nc.dram_tensor("e0", [1, P], F32R, kind="ExternalInput")
    e1d = nc.dram_tensor("e1", [1, P], F32R, kind="ExternalInput")
    out_d = nc.dram_tensor("out", [D_MODEL, SW], BF16, kind="ExternalOutput")

    with tile.TileContext(nc) as tc:
        with (
            tc.tile_pool(name="const", bufs=1) as cpool,
            tc.tile_pool(name="mats", bufs=1) as mpool,
            tc.tile_pool(name="xt", bufs=2) as xpool,
            tc.tile_pool(name="sc", bufs=2) as spool,
            tc.tile_pool(name="pt", bufs=8) as ptpool,
            tc.tile_pool(name="at", bufs=1) as apool,
            tc.tile_pool(name="ps", bufs=2, space="PSUM") as pspool,
            tc.tile_pool(name="pss", bufs=2, space="PSUM") as psspool,
            tc.tile_pool(name="dram", bufs=1, space="DRAM") as dpool,
        ):
            # ---- constants ----
            w_sb = {}
            for name, wsrc in (("q", wq), ("k", wk), ("vt", wvt)):
                t = cpool.tile([P, D_MODEL], BF16, tag=f"w{name}")
                # ACT ring: keeps the SP ring free for chunk 0's xt tiles
                nc.scalar.dma_start(out=t[:], in_=wsrc[:])
                w_sb[name] = t
            # ct/st/wo are loaded lazily (per-chunk / after chunk 0) so the
            # startup queues belong to chunk 0's xt tiles
            ct = cpool.tile([P, seq], BF16, tag="ct")
            st = cpool.tile([P, seq], BF16, tag="st")
            tri = cpool.tile([P, 128], BF16, tag="tri")
            nc.scalar.dma_start(out=tri[:], in_=trid[:])
            e0 = cpool.tile([1, P], F32R, tag="e0")
            nc.scalar.dma_start(out=e0[:], in_=e0d[:])
            e1 = cpool.tile([1, P], F32R, tag="e1")
            nc.scalar.dma_start(out=e1[:], in_=e1d[:])
            wo_sb = cpool.tile([P, KD * D_MODEL], BF16, tag="wo")

            # ---- persistent matrices ----
            qT = mpool.tile([P, seq], BF16, tag="qT")  # rows: 2 heads x 64
            kT = mpool.tile([P, seq], BF16, tag="kT")
            vnat = mpool.tile([P, KB * 130], BF16, tag="vnat")
            # ones columns (cols 64 and 129 of each 130-wide block) feed the
            # softmax denominator row of the PV matmul
            vv = vnat[:].rearrange("p (k c) -> p k c", c=130)
            nc.gpsimd.memset(vv[:, :, 64:65], 1.0)
            nc.gpsimd.memset(vv[:, :, 129:130], 1.0)

            a2a_in = dpool.tile([N_CORES, P, SW], BF16, tag="a2a_in")
            a2a_out = dpool.tile([N_CORES, P, SW], BF16, tag="a2a_out")

            # ---- warmup: preload the Exp activation table while the
            # first DMAs are in flight (real hw pays ~1.3us on first use) ----
            wz = cpool.tile([1, 128], BF16, tag="warm")
            nc.gpsimd.memset(wz[:], 0.0)
            wpt = cpool.tile([1, 128], BF16, tag="warmo")
            nc.scalar.activation(wpt[:], wz[:], EXP)

            def proj_stages(sc):
                """Q/K/V projection + rope for chunk sc as a list of stage
                closures, so the PE work can be dribbled into the previous
                chunk's attention stream (one stage per few key blocks)
                instead of landing as one ~5us block that starves ACT."""
                sl = bass.ts(sc, CH)
                xts = []

                def st_dma():
                    for k in range(KD):
                        t = xpool.tile([P, CH], BF16, tag=f"xt{k}",
                                       name=f"xt_{sc}_{k}")
                        nc.sync.dma_start(
                            out=t[:], in_=xt[128 * k:128 * (k + 1), sl]
                        )
                        xts.append(t)
                    nc.sync.dma_start(out=ct[:, sl], in_=ctab[:, sl])
                    nc.sync.dma_start(out=st[:, sl], in_=stab[:, sl])

                def st_qk(name, dst):
                    ps = pspool.tile([P, CH], F32, tag="mm",
                                     name=f"proj_{sc}_{name}")
                    for k in range(KD):
                        nc.tensor.matmul(
                            ps[:],
                            w_sb[name][:, bass.ts(k, 128)],
                            xts[k][:],
                            start=(k == 0),
                            stop=(k == KD - 1),
                        )
                    nc.vector.tensor_copy(dst[:, sl], ps[:])

                def st_rope():
                    # rope in-place on qT/kT (bf16, DVE 2x): copy the
                    # half-swapped rows (copies may cross partition
                    # offsets; tensor-tensor ops may not), then aligned ops
                    for mi, mat in ((0, qT), (1, kT)):
                        tm = spool.tile([P, CH], BF16, tag="tm",
                                        name=f"tm_{sc}_{mi}")
                        for h in (0, 1):
                            for half in (0, 1):
                                d0 = 64 * h + 32 * half
                                s0 = 64 * h + 32 * (1 - half)
                                nc.vector.tensor_copy(
                                    tm[d0:d0 + 32, :], mat[s0:s0 + 32, sl]
                                )
                        nc.vector.tensor_mul(tm[:], tm[:], st[:, sl])
                        nc.vector.tensor_mul(mat[:, sl], mat[:, sl],
                                             ct[:, sl])
                        nc.vector.tensor_add(mat[:, sl], mat[:, sl], tm[:])

                def st_v(r):
                    # V directly transposed: [128 keys, 128 dims] region
                    # (shares the "mm" tag; only cols 0:128 are used)
                    psv = pspool.tile([P, CH], F32, tag="mm",
                                      name=f"vtr_{sc}_{r}")
                    rs = bass.ts(r, 128)
                    for j in range(KD):
                        nc.tensor.matmul(
                            psv[:, 0:128],
                            xts[j][:, rs],
                            w_sb["vt"][:, bass.ts(j, 128)],
                            start=(j == 0),
                            stop=(j == KD - 1),
                        )
                    kb = sc * KBC + r
                    nc.vector.tensor_copy(
                        vnat[:, 130 * kb:130 * kb + 64], psv[:, 0:64]
                    )
                    nc.vector.tensor_copy(
                        vnat[:, 130 * kb + 65:130 * kb + 129],
                        psv[:, 64:128],
                    )

                stages = [st_dma,
                          lambda: st_qk("q", qT),
                          lambda: st_qk("k", kT),
                          st_rope]
                stages += [(lambda r=r: st_v(r)) for r in range(KBC)]
                return stages

            def emit_proj_chunk(sc):
                for f in proj_stages(sc):
                    f()

            def emit_attn_chunk(qc, inject=None, epi=None):
                """Attention for query chunk qc (needs proj chunks 0..qc).

                Per key block: S^T for both heads lands in one [128, 2*CH]
                PSUM tile ([0:CH]=h0, [CH:2CH]=h1). Diagonal-band blocks run
                FIRST; their score/exp/PV work is restricted to the valid
                column range [128j, CH), and the triangle sub-block is masked
                on GPSIMD.

                `inject` is called after the diagonal group: the next chunk's
                projection matmuls slot into the PE stream there, overlapping
                the Activation engine's exp backlog so ACT never starves at
                chunk boundaries.

                `epi` is the PREVIOUS chunk's deferred epilogue; it is
                emitted just before this chunk's first PV so the reciprocal
                chain overlaps this chunk's diagonal scores instead of
                stalling the PE at the chunk boundary. Returns this chunk's
                epilogue closure for the same treatment."""
                kbmax = (qc + 1) * KBC
                psu = {}
                for h in (0, 1):
                    psu[h] = pspool.tile([65, CH], F32, tag="u",
                                         name=f"psu_{qc}_{h}")
                kb_order = (list(range(kbmax - KBC, kbmax))
                            + list(range(0, kbmax - KBC)))

                def emit_pv(kb, ki, pt, q0):
                    for h in (0, 1):
                        nc.tensor.matmul(
                            psu[h][:, q0:CH],
                            vnat[:, 130 * kb + 65 * h:
                                 130 * kb + 65 * (h + 1)],
                            pt[:, CH * h + q0:CH * (h + 1)],
                            start=(ki == 0),
                            stop=(ki == kbmax - 1),
                        )

                DEFER = 4       # PV trails exp by this many key blocks
                pending = []    # (kb, ki, pt, q0) with PV deferred
                inject = list(inject) if inject else []
                for ki, kb in enumerate(kb_order):
                    if ki == 1 and epi is not None:
                        epi()
                        epi = None
                    # dribble the next chunk's projection stages into the
                    # PE stream: DMAs right away, one compute stage per two
                    # key blocks after the diagonal group
                    if inject and (ki == 1 or
                                   (ki >= KBC and (ki - KBC) % 2 == 0)):
                        inject.pop(0)()
                    j = kb - (kbmax - KBC)  # diag index if >= 0
                    lastkb = (ki == kbmax - 1)
                    # column-restrict diagonal blocks except the last kb of
                    # the chunk (its stop=True must cover the full psu width)
                    q0 = 128 * j if (j >= 1 and not lastkb) else 0
                    pss = psspool.tile([P, 2 * CH], F32, tag="s",
                                       name=f"sc_{qc}_{kb}")
                    for h in (0, 1):
                        nc.tensor.matmul(
                            pss[:, CH * h + q0:CH * (h + 1)],
                            kT[64 * h:64 * (h + 1), bass.ts(kb, 128)],
                            qT[64 * h:64 * (h + 1),
                               qc * CH + q0:(qc + 1) * CH],
                            start=True,
                            stop=True,
                        )
                    pt = ptpool.tile([P, 2 * CH], BF16, tag="pt",
                                     name=f"pt_{qc}_{kb}")
                    if q0:
                        for h in (0, 1):
                            nc.scalar.activation(
                                pt[:, CH * h + q0:CH * (h + 1)],
                                pss[:, CH * h + q0:CH * (h + 1)],
                                EXP,
                            )
                    else:
                        nc.scalar.activation(pt[:], pss[:], EXP)
                    if j >= 0:
                        if lastkb and j >= 1:
                            # full-width last kb on a diagonal block (qc==0):
                            # zero the columns left of the band
                            for h in (0, 1):
                                nc.gpsimd.memset(
                                    pt[:, CH * h:CH * h + 128 * j], 0.0
                                )
                        for h in (0, 1):
                            o = CH * h + 128 * j
                            nc.gpsimd.tensor_mul(
                                pt[:, o:o + 128], pt[:, o:o + 128], tri[:]
                            )
                    if parts == "attn_s":
                        continue
                    pending.append((kb, ki, pt, q0))
                    if len(pending) > DEFER:
                        emit_pv(*pending.pop(0))
                while pending and parts != "attn_s":
                    emit_pv(*pending.pop(0))
                for f in inject:
                    f()
                if parts in ("attn_s", "attn_pv"):
                    return None

                def epilogue(final=False):
                    # normalize U by the denominator row and stage the bf16
                    # payload for the AllToAll; the final chunk's chain gates
                    # the AllToAll
                    rcp = []
                    with nc.allow_low_precision(
                            reason="f32r tag only; PE rounds on read"):
                        for h in (0, 1):
                            r = spool.tile([1, CH], F32R, tag=f"rs{h}",
                                           name=f"rs_{qc}_{h}")
                            nc.vector.reciprocal(r[:], psu[h][64:65, :])
                            rcp.append(r)
                    rb = pspool.tile([P, CH], F32, tag="mm", name=f"rb_{qc}")
                    nc.tensor.matmul(rb[:], e0[:], rcp[0][:],
                                     start=True, stop=False)
                    nc.tensor.matmul(rb[:], e1[:], rcp[1][:],
                                     start=False, stop=True)
                    # copy U halves into stg (copies may remap partitions),
                    # then normalize in place with partition-aligned muls
                    stg = spool.tile([P, CH], BF16, tag="stg",
                                     name=f"stg_{qc}")
                    for h in (0, 1):
                        hs = slice(64 * h, 64 * (h + 1))
                        nc.vector.tensor_copy(stg[hs, :], psu[h][0:64, :])
                        nc.vector.tensor_mul(stg[hs, :], stg[hs, :],
                                             rb[hs, :])
                    for half in (0, 1):
                        hs = bass.ts(half, CH // 2)
                        nc.sync.dma_start(out=a2a_in[qc, :, hs],
                                          in_=stg[:, hs])

                return epilogue

            def emit_p12():
                if parts.split("_")[0] in ("dma", "proj", "rope", "noil"):
                    for sc in range(NCH):
                        emit_proj_chunk(sc)
                    if parts == "noil":
                        for sc in range(NCH):
                            emit_attn_chunk(sc)
                    return
                emit_proj_chunk(0)
                for e in range(KD):
                    nc.sync.dma_start(
                        out=wo_sb[:, bass.ts(e, D_MODEL)],
                        in_=wo[:, bass.ts(e, D_MODEL)],
                    )
                DEFER_EPI = True
                epi = None
                for qc in range(NCH):
                    nxt = qc + 1
                    ret = emit_attn_chunk(
                        qc,
                        inject=proj_stages(nxt) if nxt < NCH else None,
                        epi=epi,
                    )
                    if DEFER_EPI:
                        epi = ret
                    elif ret is not None:
                        ret()
                if epi is not None:
                    epi(final=True)

            def emit_cc():
                nc.gpsimd.collective_compute(
                    "AllToAll",
                    mybir.AluOpType.bypass,
                    replica_groups=[list(range(N_CORES))],
                    ins=[a2a_in.opt()],
                    outs=[a2a_out.opt()],
                )

            def emit_p3(cc_done):
                if not cc_done:
                    emit_cc()
                ats = []
                for i in range(N_CORES):
                    at = apool.tile([P, SW], BF16, tag=f"at{i}",
                                    name=f"at_{i}")
                    eng = nc.sync if i % 2 == 0 else nc.scalar
                    eng.dma_start(out=at[:], in_=a2a_out[i])
                    ats.append(at)
                for e in range(KD):
                    pso = pspool.tile([P, SW], F32, tag="mm",
                                      name=f"pso_{e}")
                    for i in range(N_CORES):
                        nc.tensor.matmul(
                            pso[:],
                            wo_sb[:, D_MODEL * e + 128 * i:
                                  D_MODEL * e + 128 * (i + 1)],
                            ats[i][:],
                            start=(i == 0),
                            stop=(i == N_CORES - 1),
                        )
                    ot = ptpool.tile([P, SW], BF16, tag="ot", name=f"ot_{e}")
                    # alternate the PSUM drain between DVE and ACT (both
                    # idle in phase 3) and spread out DMAs over two rings
                    if e % 2 == 0:
                        nc.vector.tensor_copy(ot[:], pso[:])
                    else:
                        nc.scalar.copy(ot[:], pso[:])
                    eng = nc.sync if e % 2 == 0 else nc.scalar
                    eng.dma_start(out=out_d[bass.ts(e, 128)], in_=ot[:])

            if p12_reps == 1:
                emit_p12()
                if parts == "full":
                    emit_cc()
            else:
                with tc.For_i(0, p12_reps, 1):
                    emit_p12()
            if parts == "full":
                for r3 in range(p3_reps):
                    emit_p3(cc_done=(p12_reps == 1 and r3 == 0))

    nc.finalize()
    return nc


def prepare_in_maps(in_features, token_positions, Wq, Wk, Wv, Wo, seq):
    """Host-side staging: shard/transform full inputs into per-core maps."""
    import ml_dtypes
    bf16 = ml_dtypes.bfloat16
    x = np.ascontiguousarray(np.asarray(in_features, dtype=np.float32)[0])
    pos = np.asarray(token_positions).reshape(-1)[:seq].astype(np.float64)

    xt = np.ascontiguousarray(x.T).astype(bf16)  # [D, S]

    # RoPE tables in rotate-half form after pair permutation.
    inv_freq = THETA ** (-np.arange(0, HEAD_DIM, 2, dtype=np.float64)
                         / HEAD_DIM)
    ang = pos[:, None] * inv_freq[None, :]  # [S, 32]
    cos = np.cos(ang).T.astype(np.float32)  # [32, S]
    sin = np.sin(ang).T.astype(np.float32)
    ctab = np.ascontiguousarray(np.tile(cos, (4, 1))).astype(bf16)
    stab = np.ascontiguousarray(
        np.concatenate([-sin, sin, -sin, sin], axis=0)
    ).astype(bf16)

    perm = np.concatenate(
        [np.arange(0, HEAD_DIM, 2), np.arange(1, HEAD_DIM, 2)]
    )  # within-head: evens then odds

    tri = np.triu(np.ones((128, 128), dtype=np.float32)).astype(bf16)
    e0_host = np.zeros((1, 128), dtype=np.float32)
    e0_host[0, 0:64] = 1.0
    e1_host = np.zeros((1, 128), dtype=np.float32)
    e1_host[0, 64:128] = 1.0

    import ml_dtypes as _mld
    WoT = np.ascontiguousarray(np.asarray(Wo, dtype=np.float32).T)  # [d, e]
    wo_packed = np.empty((128, KD * D_MODEL), dtype=np.float32)
    for e in range(KD):
        for i in range(KD):
            wo_packed[:, D_MODEL * e + 128 * i: D_MODEL * e + 128 * (i + 1)] \
                = WoT[128 * i:128 * (i + 1), 128 * e:128 * (e + 1)]
    wo_packed = wo_packed.astype(bf16)

    def pack_w(Wc):
        # Wc: [128 out, 1024 in] -> WT [1024, 128] -> [128, 8*128] k-tiled
        WT = np.ascontiguousarray(Wc.T)
        return np.ascontiguousarray(
            WT.reshape(KD, 128, 128).transpose(1, 0, 2).reshape(128, KD * 128)
        ).astype(bf16)

    in_maps = []
    for c in range(N_CORES):
        rows = slice(128 * c, 128 * (c + 1))
        Wq_r = np.asarray(Wq, dtype=np.float32)[rows].reshape(2, 64, D_MODEL)
        Wq_c = (Wq_r[:, perm, :] / math.sqrt(HEAD_DIM)).reshape(128, D_MODEL)
        Wk_r = np.asarray(Wk, dtype=np.float32)[rows].reshape(2, 64, D_MODEL)
        Wk_c = Wk_r[:, perm, :].reshape(128, D_MODEL)
        # wvt block j = WvT[128j:128(j+1), :] = Wv_core[:, 128j:...].T
        WvT = np.ascontiguousarray(
            np.asarray(Wv, dtype=np.float32)[rows].T
        )  # [1024 in, 128 out]
        wvt_packed = np.ascontiguousarray(
            WvT.reshape(KD, 128, 128).transpose(1, 0, 2).reshape(
                128, KD * 128)
        ).astype(bf16)
        in_maps.append({
            "xt": xt,
            "wq": pack_w(Wq_c),
            "wk": pack_w(Wk_c),
            "wvt": wvt_packed,
            "wo": wo_packed,
            "ctab": ctab,
            "stab": stab,
            "tri": tri,
            "e0": e0_host,
            "e1": e1_host,
        })
    return in_maps


_BUILD_CACHE = {}


def _get_nc(seq, p12_reps=1, p3_reps=1, parts="full"):
    key = (seq, p12_reps, p3_reps, parts)
    if key not in _BUILD_CACHE:
        _BUILD_CACHE[key] = build(seq, p12_reps, p3_reps, parts)
    return _BUILD_CACHE[key]


def postprocess(results, seq, in_dtype):
    SW = seq // N_CORES
    out = np.empty((seq, D_MODEL), dtype=np.float32)
    for c in range(N_CORES):
        out[SW * c:SW * (c + 1), :] = results[c]["out"].T
    return out.reshape(1, seq, D_MODEL).astype(in_dtype)


def kernel(in_features, token_positions, Wq, Wk, Wv, Wo):
    in_dtype = np.asarray(in_features).dtype
    B, S, D = np.asarray(in_features).shape
    assert B == 1 and D == D_MODEL

    nc = _get_nc(S)
    in_maps = prepare_in_maps(in_features, token_positions, Wq, Wk, Wv, Wo, S)
    res = run_bass_kernel_spmd(nc, in_maps, list(range(N_CORES)), trace=False)
    return postprocess(res.results, S, in_dtype)


# revision 73
# speedup vs baseline: 1.3003x; 1.0596x over previous
"""Multi-head self-attention with RoPE, sharded over 8 TRN2 NeuronCores.

Sharding: tensor-parallel over heads (2 heads/core) for QKV projections and
attention; one AllToAll redistributes normalized attention outputs from
head-sharded to sequence-sharded so each core computes 1/8 of the output
projection columns.

v2 design notes (vs the v1 baseline, sim 362us -> 262us):
- V is projected directly in transposed orientation (keys on partitions) by
  swapping matmul operand roles, removing the PE transposes + DVE extracts.
- RoPE runs in bf16 on qT/kT in place (DVE 2x mode); cos/sin tables ship as
  bf16 and are DMA'd per chunk so startup isn't gated on 4MB of tables.
- Causal masking runs on the idle GPSIMD(Pool) engine with a single [128,128]
  triangle tile; score/exp/PV work on diagonal blocks is column-restricted to
  the valid region.
- The next chunk's projection is emitted as small stages dribbled between
  attention key blocks (and each chunk's epilogue is deferred into the next
  chunk), so the in-order PE queue keeps the Activation engine's exp stream
  fed at chunk boundaries; PV trails exp by DEFER key blocks for elasticity.
- Attention outputs are normalized on the producing core (reciprocal +
  broadcast-matmul), so the AllToAll payload is a single bf16 [8,128,512]
  buffer: half the bytes and one collective instead of two zero-padded ones.
- Out-projection weights are bf16 and SBUF-resident before phase 3; the
  output is written bf16 and widened on the host.
- fp8 was tried for scores/PV/payload/out-proj and REJECTED: with random-sign
  contractions the ~5% e4m3 element error lands directly on the output
  (rel err 4e-2 > the 2e-2 gate). Exp is Activation-engine-only on TRN2
  (verifier NCC_IBIR606), so the ~145us exp stream is the late-phase floor.

Hardcoded problem shape: B=1, S=4096, D=1024, H=16, hd=64, theta=10000.
"""

import math

import numpy as np

import concourse.bass as bass
import concourse.mybir as mybir
import concourse.tile as tile
from concourse import bacc
from concourse.bass_utils import run_bass_kernel_spmd

N_CORES = 8
D_MODEL = 1024
NUM_HEADS = 16
HEAD_DIM = 64
THETA = 10000.0
P = 128  # partitions; also = 2 heads x 64 dims per core
KD = D_MODEL // 128  # 8 contraction tiles for the projections

F32 = mybir.dt.float32
F32R = mybir.dt.float32r
BF16 = mybir.dt.bfloat16
FP8 = mybir.dt.float8e4
EXP = mybir.ActivationFunctionType.Exp
DR = mybir.MatmulPerfMode.DoubleRow


def build(seq: int, p12_reps: int = 1, p3_reps: int = 1, parts: str = "full"):
    """Build the SPMD Bass program for sequence length `seq`.

    p12_reps > 1 wraps phases 1+2 (projections + attention) in an on-device
    For_i loop; p3_reps > 1 unrolls phase 3 (A2A + out-proj) — both exist
    for wall-clock timing above the axon dispatch floor."""
    CH = min(512, seq)          # free-dim chunk for matmuls / PSUM banks
    NCH = seq // CH             # number of seq chunks
    KB = seq // 128             # key blocks
    KBC = CH // 128             # key blocks per chunk (4 at CH=512)
    SW = seq // N_CORES         # per-core output seq shard (== CH here)

    nc = bacc.Bacc("TRN2", num_devices=N_CORES)

    xt = nc.dram_tensor("xt", [D_MODEL, seq], BF16, kind="ExternalInput")
    wq = nc.dram_tensor("wq", [P, D_MODEL], BF16, kind="ExternalInput")
    wk = nc.dram_tensor("wk", [P, D_MODEL], BF16, kind="ExternalInput")
    wvt = nc.dram_tensor("wvt", [P, D_MODEL], BF16, kind="ExternalInput")
    wo = nc.dram_tensor("wo", [P, KD * D_MODEL], BF16, kind="ExternalInput")
    ctab = nc.dram_tensor("ctab", [P, seq], BF16, kind="ExternalInput")
    stab = nc.dram_tensor("stab", [P, seq], BF16, kind="ExternalInput")
    trid = nc.dram_tensor("tri", [P, 128], BF16, kind="ExternalInput")
    e0d = nc.dram_tensor("e0", [1, P], F32R, kind="ExternalInput")
    e1d = nc.dram_tensor("e1", [1, P], F32R, kind="ExternalInput")
    out_d = nc.dram_tensor("out", [D_MODEL, SW], BF16, kind="ExternalOutput")

    with tile.TileContext(nc) as tc:
        with (
            tc.tile_pool(name="const", bufs=1) as cpool,
            tc.tile_pool(name="mats", bufs=1) as mpool,
            tc.tile_pool(name="xt", bufs=2) as xpool,
            tc.tile_pool(name="sc", bufs=2) as spool,
            tc.tile_pool(name="pt", bufs=8) as ptpool,
            tc.tile_pool(name="at", bufs=1) as apool,
            tc.tile_pool(name="ps", bufs=2, space="PSUM") as pspool,
            tc.tile_pool(name="pss", bufs=2, space="PSUM") as psspool,
            tc.tile_pool(name="dram", bufs=1, space="DRAM") as dpool,
        ):
            # ---- constants ----
            w_sb = {}
            for name, wsrc in (("q", wq), ("k", wk), ("vt", wvt)):
                t = cpool.tile([P, D_MODEL], BF16, tag=f"w{name}")
                # ACT ring: keeps the SP ring free for chunk 0's xt tiles
                nc.scalar.dma_start(out=t[:], in_=wsrc[:])
                w_sb[name] = t
            # ct/st/wo are loaded lazily (per-chunk / after chunk 0) so the
            # startup queues belong to chunk 0's xt tiles
            ct = cpool.tile([P, seq], BF16, tag="ct")
            st = cpool.tile([P, seq], BF16, tag="st")
            tri = cpool.tile([P, 128], BF16, tag="tri")
            nc.scalar.dma_start(out=tri[:], in_=trid[:])
            e0 = cpool.tile([1, P], F32R, tag="e0")
            nc.scalar.dma_start(out=e0[:], in_=e0d[:])
            e1 = cpool.tile([1, P], F32R, tag="e1")
            nc.scalar.dma_start(out=e1[:], in_=e1d[:])
            wo_sb = cpool.tile([P, KD * D_MODEL], BF16, tag="wo")

            # ---- persistent matrices ----
            qT = mpool.tile([P, seq], BF16, tag="qT")  # rows: 2 heads x 64
            kT = mpool.tile([P, seq], BF16, tag="kT")
            vnat = mpool.tile([P, KB * 130], BF16, tag="vnat")
            # ones columns (cols 64 and 129 of each 130-wide block) feed the
            # softmax denominator row of the PV matmul
            vv = vnat[:].rearrange("p (k c) -> p k c", c=130)
            nc.gpsimd.memset(vv[:, :, 64:65], 1.0)
            nc.gpsimd.memset(vv[:, :, 129:130], 1.0)

            a2a_in = dpool.tile([N_CORES, P, SW], BF16, tag="a2a_in")
            a2a_out = dpool.tile([N_CORES, P, SW], BF16, tag="a2a_out")

            # ---- warmup: preload the Exp activation table while the
            # first DMAs are in flight (real hw pays ~1.3us on first use) ----
            wz = cpool.tile([1, 128], BF16, tag="warm")
            nc.gpsimd.memset(wz[:], 0.0)
            wpt = cpool.tile([1, 128], BF16, tag="warmo")
            nc.scalar.activation(wpt[:], wz[:], EXP)

            def proj_stages(sc):
                """Q/K/V projection + rope for chunk sc as a list of stage
                closures, so the PE work can be dribbled into the previous
                chunk's attention stream (one stage per few key blocks)
                instead of landing as one ~5us block that starves ACT."""
                sl = bass.ts(sc, CH)
                xts = []

                def st_dma():
                    for k in range(KD):
                        t = xpool.tile([P, CH], BF16, tag=f"xt{k}",
                                       name=f"xt_{sc}_{k}")
                        nc.sync.dma_start(
                            out=t[:], in_=xt[128 * k:128 * (k + 1), sl]
                        )
                        xts.append(t)
                    nc.sync.dma_start(out=ct[:, sl], in_=ctab[:, sl])
                    nc.sync.dma_start(out=st[:, sl], in_=stab[:, sl])

                def st_qk(name, dst):
                    ps = pspool.tile([P, CH], F32, tag="mm",
                                     name=f"proj_{sc}_{name}")
                    for k in range(KD):
                        nc.tensor.matmul(
                            ps[:],
                            w_sb[name][:, bass.ts(k, 128)],
                            xts[k][:],
                            start=(k == 0),
                            stop=(k == KD - 1),
                        )
                    nc.vector.tensor_copy(dst[:, sl], ps[:])

                def st_rope():
                    # rope in-place on qT/kT (bf16, DVE 2x): copy the
                    # half-swapped rows (copies may cross partition
                    # offsets; tensor-tensor ops may not), then aligned ops
                    for mi, mat in ((0, qT), (1, kT)):
                        tm = spool.tile([P, CH], BF16, tag="tm",
                                        name=f"tm_{sc}_{mi}")
                        for h in (0, 1):
                            for half in (0, 1):
                                d0 = 64 * h + 32 * half
                                s0 = 64 * h + 32 * (1 - half)
                                nc.vector.tensor_copy(
                                    tm[d0:d0 + 32, :], mat[s0:s0 + 32, sl]
                                )
                        nc.vector.tensor_mul(tm[:], tm[:], st[:, sl])
                        nc.vector.tensor_mul(mat[:, sl], mat[:, sl],
                                             ct[:, sl])
                        nc.vector.tensor_add(mat[:, sl], mat[:, sl], tm[:])

                def st_v(r):
                    # V directly transposed: [128 keys, 128 dims] region
                    # (shares the "mm" tag; only cols 0:128 are used)
                    psv = pspool.tile([P, CH], F32, tag="mm",
                                      name=f"vtr_{sc}_{r}")
                    rs = bass.ts(r, 128)
                    for j in range(KD):
                        nc.tensor.matmul(
                            psv[:, 0:128],
                            xts[j][:, rs],
                            w_sb["vt"][:, bass.ts(j, 128)],
                            start=(j == 0),
                            stop=(j == KD - 1),
                        )
                    kb = sc * KBC + r
                    nc.vector.tensor_copy(
                        vnat[:, 130 * kb:130 * kb + 64], psv[:, 0:64]
                    )
                    nc.vector.tensor_copy(
                        vnat[:, 130 * kb + 65:130 * kb + 129],
                        psv[:, 64:128],
                    )

                stages = [st_dma,
                          lambda: st_qk("q", qT),
                          lambda: st_qk("k", kT),
                          st_rope]
                stages += [(lambda r=r: st_v(r)) for r in range(KBC)]
                return stages

            def emit_proj_chunk(sc):
                for f in proj_stages(sc):
                    f()

            def emit_attn_chunk(qc, inject=None, epi=None):
                """Attention for query chunk qc (needs proj chunks 0..qc).

                Per key block: S^T for both heads lands in one [128, 2*CH]
                PSUM tile ([0:CH]=h0, [CH:2CH]=h1). Diagonal-band blocks run
                FIRST; their score/exp/PV work is restricted to the valid
                column range [128j, CH), and the triangle sub-block is masked
                on GPSIMD.

                `inject` is called after the diagonal group: the next chunk's
                projection matmuls slot into the PE stream there, overlapping
                the Activation engine's exp backlog so ACT never starves at
                chunk boundaries.

                `epi` is the PREVIOUS chunk's deferred epilogue; it is
                emitted just before this chunk's first PV so the reciprocal
                chain overlaps this chunk's diagonal scores instead of
                stalling the PE at the chunk boundary. Returns this chunk's
                epilogue closure for the same treatment."""
                kbmax = (qc + 1) * KBC
                psu = {}
                for h in (0, 1):
                    psu[h] = pspool.tile([65, CH], F32, tag="u",
                                         name=f"psu_{qc}_{h}")
                kb_order = (list(range(kbmax - KBC, kbmax))
                            + list(range(0, kbmax - KBC)))

                def emit_pv(kb, ki, pt, q0):
                    for h in (0, 1):
                        nc.tensor.matmul(
                            psu[h][:, q0:CH],
                            vnat[:, 130 * kb + 65 * h:
                                 130 * kb + 65 * (h + 1)],
                            pt[:, CH * h + q0:CH * (h + 1)],
                            start=(ki == 0),
                            stop=(ki == kbmax - 1),
                        )

                DEFER = 4       # PV trails exp by this many key blocks
                pending = []    # (kb, ki, pt, q0) with PV deferred
                inject = list(inject) if inject else []
                for ki, kb in enumerate(kb_order):
                    if ki == 1 and epi is not None:
                        epi()
                        epi = None
                    # dribble the next chunk's projection stages into the
                    # PE stream: DMAs right away, one compute stage per two
                    # key blocks after the diagonal group
                    if inject and (ki == 1 or
                                   (ki >= KBC and (ki - KBC) % 2 == 0)):
                        inject.pop(0)()
                    j = kb - (kbmax - KBC)  # diag index if >= 0
                    lastkb = (ki == kbmax - 1)
                    # column-restrict diagonal blocks except the last kb of
                    # the chunk (its stop=True must cover the full psu width)
                    q0 = 128 * j if (j >= 1 and not lastkb) else 0
                    pss = psspool.tile([P, 2 * CH], F32, tag="s",
                                       name=f"sc_{qc}_{kb}")
                    for h in (0, 1):
                        nc.tensor.matmul(
                            pss[:, CH * h + q0:CH * (h + 1)],
                            kT[64 * h:64 * (h + 1), bass.ts(kb, 128)],
                            qT[64 * h:64 * (h + 1),
                               qc * CH + q0:(qc + 1) * CH],
                            start=True,
                            stop=True,
                        )
                    pt = ptpool.tile([P, 2 * CH], BF16, tag="pt",
                                     name=f"pt_{qc}_{kb}")
                    if q0:
                        for h in (0, 1):
                            nc.scalar.activation(
                                pt[:, CH * h + q0:CH * (h + 1)],
                                pss[:, CH * h + q0:CH * (h + 1)],
                                EXP,
                            )
                    else:
                        nc.scalar.activation(pt[:], pss[:], EXP)
                    if j >= 0:
                        if lastkb and j >= 1:
                            # full-width last kb on a diagonal block (qc==0):
                            # zero the columns left of the band
                            for h in (0, 1):
                                nc.gpsimd.memset(
                                    pt[:, CH * h:CH * h + 128 * j], 0.0
                                )
                        for h in (0, 1):
                            o = CH * h + 128 * j
                            nc.gpsimd.tensor_mul(
                                pt[:, o:o + 128], pt[:, o:o + 128], tri[:]
                            )
                    if parts == "attn_s":
                        continue
                    pending.append((kb, ki, pt, q0))
                    if len(pending) > DEFER:
                        emit_pv(*pending.pop(0))
                while pending and parts != "attn_s":
                    emit_pv(*pending.pop(0))
                for f in inject:
                    f()
                if parts in ("attn_s", "attn_pv"):
                    return None

                def epilogue(final=False):
                    # normalize U by the denominator row and stage the bf16
                    # payload for the AllToAll; the final chunk's chain gates
                    # the AllToAll
                    rcp = []
                    with nc.allow_low_precision(
                            reason="f32r tag only; PE rounds on read"):
                        for h in (0, 1):
                            r = spool.tile([1, CH], F32R, tag=f"rs{h}",
                                           name=f"rs_{qc}_{h}")
                            nc.vector.reciprocal(r[:], psu[h][64:65, :])
                            rcp.append(r)
                    rb = pspool.tile([P, CH], F32, tag="mm", name=f"rb_{qc}")
                    nc.tensor.matmul(rb[:], e0[:], rcp[0][:],
                                     start=True, stop=False)
                    nc.tensor.matmul(rb[:], e1[:], rcp[1][:],
                                     start=False, stop=True)
                    # copy U halves into stg (copies may remap partitions),
                    # then normalize in place with partition-aligned muls
                    stg = spool.tile([P, CH], BF16, tag="stg",
                                     name=f"stg_{qc}")
                    # copy U halves into stg first: tensor-tensor ops may
                    # read at most one PSUM operand and need equal input
                    # base partitions, so the muls read stg(SB) x rb(PSUM)
                    for h in (0, 1):
                        hs = slice(64 * h, 64 * (h + 1))
                        nc.vector.tensor_copy(stg[hs, :], psu[h][0:64, :])
                        nc.vector.tensor_mul(stg[hs, :], stg[hs, :],
                                             rb[hs, :])
                    for half in (0, 1):
                        hs = bass.ts(half, CH // 2)
                        nc.sync.dma_start(out=a2a_in[qc, :, hs],
                                          in_=stg[:, hs])

                return epilogue

            def emit_p12():
                if parts.split("_")[0] in ("dma", "proj", "rope", "noil"):
                    for sc in range(NCH):
                        emit_proj_chunk(sc)
                    if parts == "noil":
                        for sc in range(NCH):
                            emit_attn_chunk(sc)
                    return
                emit_proj_chunk(0)
                for e in range(KD):
                    nc.sync.dma_start(
                        out=wo_sb[:, bass.ts(e, D_MODEL)],
                        in_=wo[:, bass.ts(e, D_MODEL)],
                    )
                DEFER_EPI = True
                epi = None
                for qc in range(NCH):
                    nxt = qc + 1
                    ret = emit_attn_chunk(
                        qc,
                        inject=proj_stages(nxt) if nxt < NCH else None,
                        epi=epi,
                    )
                    if DEFER_EPI:
                        epi = ret
                    elif ret is not None:
                        ret()
                if epi is not None:
                    epi(final=True)

            def emit_cc():
                nc.gpsimd.collective_compute(
                    "AllToAll",
                    mybir.AluOpType.bypass,
                    replica_groups=[list(range(N_CORES))],
                    ins=[a2a_in.opt()],
                    outs=[a2a_out.opt()],
                )

            def emit_p3(cc_done):
                if not cc_done:
                    emit_cc()
                ats = []
                for i in range(N_CORES):
                    at = apool.tile([P, SW], BF16, tag=f"at{i}",
                                    name=f"at_{i}")
                    eng = nc.sync if i % 2 == 0 else nc.scalar
                    eng.dma_start(out=at[:], in_=a2a_out[i])
                    ats.append(at)
                for e in range(KD):
                    pso = pspool.tile([P, SW], F32, tag="mm",
                                      name=f"pso_{e}")
                    for i in range(N_CORES):
                        nc.tensor.matmul(
                            pso[:],
                            wo_sb[:, D_MODEL * e + 128 * i:
                                  D_MODEL * e + 128 * (i + 1)],
                            ats[i][:],
                            start=(i == 0),
                            stop=(i == N_CORES - 1),
                        )
                    ot = ptpool.tile([P, SW], BF16, tag="ot", name=f"ot_{e}")
                    # alternate the PSUM drain between DVE and ACT (both
                    # idle in phase 3) and spread out DMAs over two rings
                    if e % 2 == 0:
                        nc.vector.tensor_copy(ot[:], pso[:])
                    else:
                        nc.scalar.copy(ot[:], pso[:])
                    eng = nc.sync if e % 2 == 0 else nc.scalar
                    eng.dma_start(out=out_d[bass.ts(e, 128)], in_=ot[:])

            if p12_reps == 1:
                emit_p12()
                if parts == "full":
                    emit_cc()
            else:
                with tc.For_i(0, p12_reps, 1):
                    emit_p12()
            if parts == "full":
                for r3 in range(p3_reps):
                    emit_p3(cc_done=(p12_reps == 1 and r3 == 0))

    nc.finalize()
    return nc


def prepare_in_maps(in_features, token_positions, Wq, Wk, Wv, Wo, seq):
    """Host-side staging: shard/transform full inputs into per-core maps."""
    import ml_dtypes
    bf16 = ml_dtypes.bfloat16
    x = np.ascontiguousarray(np.asarray(in_features, dtype=np.float32)[0])
    pos = np.asarray(token_positions).reshape(-1)[:seq].astype(np.float64)

    xt = np.ascontiguousarray(x.T).astype(bf16)  # [D, S]

    # RoPE tables in rotate-half form after pair permutation.
    inv_freq = THETA ** (-np.arange(0, HEAD_DIM, 2, dtype=np.float64)
                         / HEAD_DIM)
    ang = pos[:, None] * inv_freq[None, :]  # [S, 32]
    cos = np.cos(ang).T.astype(np.float32)  # [32, S]
    sin = np.sin(ang).T.astype(np.float32)
    ctab = np.ascontiguousarray(np.tile(cos, (4, 1))).astype(bf16)
    stab = np.ascontiguousarray(
        np.concatenate([-sin, sin, -sin, sin], axis=0)
    ).astype(bf16)

    perm = np.concatenate(
        [np.arange(0, HEAD_DIM, 2), np.arange(1, HEAD_DIM, 2)]
    )  # within-head: evens then odds

    tri = np.triu(np.ones((128, 128), dtype=np.float32)).astype(bf16)
    e0_host = np.zeros((1, 128), dtype=np.float32)
    e0_host[0, 0:64] = 1.0
    e1_host = np.zeros((1, 128), dtype=np.float32)
    e1_host[0, 64:128] = 1.0

    import ml_dtypes as _mld
    WoT = np.ascontiguousarray(np.asarray(Wo, dtype=np.float32).T)  # [d, e]
    wo_packed = np.empty((128, KD * D_MODEL), dtype=np.float32)
    for e in range(KD):
        for i in range(KD):
            wo_packed[:, D_MODEL * e + 128 * i: D_MODEL * e + 128 * (i + 1)] \
                = WoT[128 * i:128 * (i + 1), 128 * e:128 * (e + 1)]
    wo_packed = wo_packed.astype(bf16)

    def pack_w(Wc):
        # Wc: [128 out, 1024 in] -> WT [1024, 128] -> [128, 8*128] k-tiled
        WT = np.ascontiguousarray(Wc.T)
        return np.ascontiguousarray(
            WT.reshape(KD, 128, 128).transpose(1, 0, 2).reshape(128, KD * 128)
        ).astype(bf16)

    in_maps = []
    for c in range(N_CORES):
        rows = slice(128 * c, 128 * (c + 1))
        Wq_r = np.asarray(Wq, dtype=np.float32)[rows].reshape(2, 64, D_MODEL)
        Wq_c = (Wq_r[:, perm, :] / math.sqrt(HEAD_DIM)).reshape(128, D_MODEL)
        Wk_r = np.asarray(Wk, dtype=np.float32)[rows].reshape(2, 64, D_MODEL)
        Wk_c = Wk_r[:, perm, :].reshape(128, D_MODEL)
        # wvt block j = WvT[128j:128(j+1), :] = Wv_core[:, 128j:...].T
        WvT = np.ascontiguousarray(
            np.asarray(Wv, dtype=np.float32)[rows].T
        )  # [1024 in, 128 out]
        wvt_packed = np.ascontiguousarray(
            WvT.reshape(KD, 128, 128).transpose(1, 0, 2).reshape(
                128, KD * 128)
        ).astype(bf16)
        in_maps.append({
            "xt": xt,
            "wq": pack_w(Wq_c),
            "wk": pack_w(Wk_c),
            "wvt": wvt_packed,
            "wo": wo_packed,
            "ctab": ctab,
            "stab": stab,
            "tri": tri,
            "e0": e0_host,
            "e1": e1_host,
        })
    return in_maps


_BUILD_CACHE = {}


def _get_nc(seq, p12_reps=1, p3_reps=1, parts="full"):
    key = (seq, p12_reps, p3_reps, parts)
    if key not in _BUILD_CACHE:
        _BUILD_CACHE[key] = build(seq, p12_reps, p3_reps, parts)
    return _BUILD_CACHE[key]


def postprocess(results, seq, in_dtype):
    SW = seq // N_CORES
    out = np.empty((seq, D_MODEL), dtype=np.float32)
    for c in range(N_CORES):
        out[SW * c:SW * (c + 1), :] = results[c]["out"].T
    return out.reshape(1, seq, D_MODEL).astype(in_dtype)


def kernel(in_features, token_positions, Wq, Wk, Wv, Wo):
    in_dtype = np.asarray(in_features).dtype
    B, S, D = np.asarray(in_features).shape
    assert B == 1 and D == D_MODEL

    nc = _get_nc(S)
    in_maps = prepare_in_maps(in_features, token_positions, Wq, Wk, Wv, Wo, S)
    res = run_bass_kernel_spmd(nc, in_maps, list(range(N_CORES)), trace=False)
    return postprocess(res.results, S, in_dtype)


# revision 76
# speedup vs baseline: 1.5804x; 1.2154x over previous
"""Multi-head self-attention with RoPE, sharded over 8 TRN2 NeuronCores.

Sharding: tensor-parallel over heads (2 heads/core) for QKV projections and
attention; one AllToAll redistributes normalized attention outputs from
head-sharded to sequence-sharded so each core computes 1/8 of the output
projection columns.

v2 design notes (vs the v1 baseline, sim 362us -> 262us):
- V is projected directly in transposed orientation (keys on partitions) by
  swapping matmul operand roles, removing the PE transposes + DVE extracts.
- RoPE runs in bf16 on qT/kT in place (DVE 2x mode); cos/sin tables ship as
  bf16 and are DMA'd per chunk so startup isn't gated on 4MB of tables.
- Causal masking runs on the idle GPSIMD(Pool) engine with a single [128,128]
  triangle tile; score/exp/PV work on diagonal blocks is column-restricted to
  the valid region.
- The next chunk's projection is emitted as small stages dribbled between
  attention key blocks (and each chunk's epilogue is deferred into the next
  chunk), so the in-order PE queue keeps the Activation engine's exp stream
  fed at chunk boundaries; PV trails exp by DEFER key blocks for elasticity.
- Attention outputs are normalized on the producing core (reciprocal +
  broadcast-matmul), so the AllToAll payload is a single bf16 [8,128,512]
  buffer: half the bytes and one collective instead of two zero-padded ones.
- Out-projection weights are bf16 and SBUF-resident before phase 3; the
  output is written bf16 and widened on the host.
- fp8 was tried for scores/PV/payload/out-proj and REJECTED: with random-sign
  contractions the ~5% e4m3 element error lands directly on the output
  (rel err 4e-2 > the 2e-2 gate). Exp is Activation-engine-only on TRN2
  (verifier NCC_IBIR606), so the ~145us exp stream is the late-phase floor.

Hardcoded problem shape: B=1, S=4096, D=1024, H=16, hd=64, theta=10000.
"""

import math

import numpy as np

import concourse.bass as bass
import concourse.mybir as mybir
import concourse.tile as tile
from concourse import bacc
from concourse.bass_utils import run_bass_kernel_spmd

N_CORES = 8
D_MODEL = 1024
NUM_HEADS = 16
HEAD_DIM = 64
THETA = 10000.0
P = 128  # partitions; also = 2 heads x 64 dims per core
KD = D_MODEL // 128  # 8 contraction tiles for the projections

F32 = mybir.dt.float32
F32R = mybir.dt.float32r
BF16 = mybir.dt.bfloat16
FP8 = mybir.dt.float8e4
EXP = mybir.ActivationFunctionType.Exp
DR = mybir.MatmulPerfMode.DoubleRow


def build(seq: int, p12_reps: int = 1, p3_reps: int = 1, parts: str = "full"):
    """Build the SPMD Bass program for sequence length `seq`.

    p12_reps > 1 wraps phases 1+2 (projections + attention) in an on-device
    For_i loop; p3_reps > 1 unrolls phase 3 (A2A + out-proj) — both exist
    for wall-clock timing above the axon dispatch floor."""
    CH = min(512, seq)          # free-dim chunk for matmuls / PSUM banks
    NCH = seq // CH             # number of seq chunks
    KB = seq // 128             # key blocks
    KBC = CH // 128             # key blocks per chunk (4 at CH=512)
    SW = seq // N_CORES         # per-core output seq shard (== CH here)

    nc = bacc.Bacc("TRN2", num_devices=N_CORES)

    xt = nc.dram_tensor("xt", [D_MODEL, seq], BF16, kind="ExternalInput")
    wq = nc.dram_tensor("wq", [P, D_MODEL], BF16, kind="ExternalInput")
    wk = nc.dram_tensor("wk", [P, D_MODEL], BF16, kind="ExternalInput")
    wvt = nc.dram_tensor("wvt", [P, D_MODEL], BF16, kind="ExternalInput")
    wo = nc.dram_tensor("wo", [P, KD * D_MODEL], BF16, kind="ExternalInput")
    ctab = nc.dram_tensor("ctab", [P, seq], BF16, kind="ExternalInput")
    stab = nc.dram_tensor("stab", [P, seq], BF16, kind="ExternalInput")
    trid = nc.dram_tensor("tri", [P, 128], BF16, kind="ExternalInput")
    e0d = nc.dram_tensor("e0", [1, P], F32R, kind="ExternalInput")
    e1d = nc.dram_tensor("e1", [1, P], F32R, kind="ExternalInput")
    out_d = nc.dram_tensor("out", [D_MODEL, SW], BF16, kind="ExternalOutput")

    with tile.TileContext(nc) as tc:
        with (
            tc.tile_pool(name="const", bufs=1) as cpool,
            tc.tile_pool(name="mats", bufs=1) as mpool,
            tc.tile_pool(name="xt", bufs=2) as xpool,
            tc.tile_pool(name="sc", bufs=2) as spool,
            tc.tile_pool(name="pt", bufs=8) as ptpool,
            tc.tile_pool(name="at", bufs=1) as apool,
            tc.tile_pool(name="ps", bufs=2, space="PSUM") as pspool,
            tc.tile_pool(name="pss", bufs=2, space="PSUM") as psspool,
            tc.tile_pool(name="dram", bufs=1, space="DRAM") as dpool,
        ):
            # ---- constants ----
            w_sb = {}
            for name, wsrc in (("q", wq), ("k", wk), ("vt", wvt)):
                t = cpool.tile([P, D_MODEL], BF16, tag=f"w{name}")
                # ACT ring: keeps the SP ring free for chunk 0's xt tiles
                nc.scalar.dma_start(out=t[:], in_=wsrc[:])
                w_sb[name] = t
            # ct/st/wo are loaded lazily (per-chunk / after chunk 0) so the
            # startup queues belong to chunk 0's xt tiles
            ct = cpool.tile([P, seq], BF16, tag="ct")
            st = cpool.tile([P, seq], BF16, tag="st")
            tri = cpool.tile([P, 128], BF16, tag="tri")
            nc.scalar.dma_start(out=tri[:], in_=trid[:])
            e0 = cpool.tile([1, P], F32R, tag="e0")
            nc.scalar.dma_start(out=e0[:], in_=e0d[:])
            e1 = cpool.tile([1, P], F32R, tag="e1")
            nc.scalar.dma_start(out=e1[:], in_=e1d[:])
            wo_sb = cpool.tile([P, KD * D_MODEL], BF16, tag="wo")

            # ---- persistent matrices ----
            qT = mpool.tile([P, seq], BF16, tag="qT")  # rows: 2 heads x 64
            kT = mpool.tile([P, seq], BF16, tag="kT")
            vnat = mpool.tile([P, KB * 130], BF16, tag="vnat")
            # ones columns (cols 64 and 129 of each 130-wide block) feed the
            # softmax denominator row of the PV matmul
            vv = vnat[:].rearrange("p (k c) -> p k c", c=130)
            nc.gpsimd.memset(vv[:, :, 64:65], 1.0)
            nc.gpsimd.memset(vv[:, :, 129:130], 1.0)

            a2a_in = dpool.tile([N_CORES, P, SW], BF16, tag="a2a_in")
            a2a_out = dpool.tile([N_CORES, P, SW], BF16, tag="a2a_out")

            # ---- warmup: preload the Exp activation table while the
            # first DMAs are in flight (real hw pays ~1.3us on first use) ----
            wz = cpool.tile([1, 128], BF16, tag="warm")
            nc.gpsimd.memset(wz[:], 0.0)
            wpt = cpool.tile([1, 128], BF16, tag="warmo")
            nc.scalar.activation(wpt[:], wz[:], EXP)

            def proj_stages(sc):
                """Q/K/V projection + rope for chunk sc as a list of stage
                closures, so the PE work can be dribbled into the previous
                chunk's attention stream (one stage per few key blocks)
                instead of landing as one ~5us block that starves ACT."""
                sl = bass.ts(sc, CH)
                xts = []

                def st_dma():
                    for k in range(KD):
                        t = xpool.tile([P, CH], BF16, tag=f"xt{k}",
                                       name=f"xt_{sc}_{k}")
                        nc.sync.dma_start(
                            out=t[:], in_=xt[128 * k:128 * (k + 1), sl]
                        )
                        xts.append(t)
                    # chunk 0 is startup-critical: its tables ride the ACT
                    # ring (idle until the first exp) so the SP ring is
                    # pure xt; later chunks have slack on SP
                    teng = nc.scalar if sc == 0 else nc.sync
                    teng.dma_start(out=ct[:, sl], in_=ctab[:, sl])
                    teng.dma_start(out=st[:, sl], in_=stab[:, sl])

                def st_qk(name, dst):
                    ps = pspool.tile([P, CH], F32, tag="mm",
                                     name=f"proj_{sc}_{name}")
                    for k in range(KD):
                        nc.tensor.matmul(
                            ps[:],
                            w_sb[name][:, bass.ts(k, 128)],
                            xts[k][:],
                            start=(k == 0),
                            stop=(k == KD - 1),
                        )
                    nc.vector.tensor_copy(dst[:, sl], ps[:])

                def st_rope():
                    # rope in-place on qT/kT (bf16, DVE 2x): copy the
                    # half-swapped rows (copies may cross partition
                    # offsets; tensor-tensor ops may not), then aligned ops
                    for mi, mat in ((0, qT), (1, kT)):
                        tm = spool.tile([P, CH], BF16, tag="tm",
                                        name=f"tm_{sc}_{mi}")
                        for h in (0, 1):
                            for half in (0, 1):
                                d0 = 64 * h + 32 * half
                                s0 = 64 * h + 32 * (1 - half)
                                nc.vector.tensor_copy(
                                    tm[d0:d0 + 32, :], mat[s0:s0 + 32, sl]
                                )
                        nc.vector.tensor_mul(tm[:], tm[:], st[:, sl])
                        nc.vector.tensor_mul(mat[:, sl], mat[:, sl],
                                             ct[:, sl])
                        nc.vector.tensor_add(mat[:, sl], mat[:, sl], tm[:])

                def st_v(r):
                    # V directly transposed: [128 keys, 128 dims] region
                    # (shares the "mm" tag; only cols 0:128 are used)
                    psv = pspool.tile([P, CH], F32, tag="mm",
                                      name=f"vtr_{sc}_{r}")
                    rs = bass.ts(r, 128)
                    for j in range(KD):
                        nc.tensor.matmul(
                            psv[:, 0:128],
                            xts[j][:, rs],
                            w_sb["vt"][:, bass.ts(j, 128)],
                            start=(j == 0),
                            stop=(j == KD - 1),
                        )
                    kb = sc * KBC + r
                    nc.vector.tensor_copy(
                        vnat[:, 130 * kb:130 * kb + 64], psv[:, 0:64]
                    )
                    nc.vector.tensor_copy(
                        vnat[:, 130 * kb + 65:130 * kb + 129],
                        psv[:, 64:128],
                    )

                stages = [st_dma,
                          lambda: st_qk("q", qT),
                          lambda: st_qk("k", kT),
                          st_rope]
                stages += [(lambda r=r: st_v(r)) for r in range(KBC)]
                return stages

            def emit_proj_chunk(sc):
                for f in proj_stages(sc):
                    f()

            def emit_attn_chunk(qc, inject=None, epi=None):
                """Attention for query chunk qc (needs proj chunks 0..qc).

                Per key block: S^T for both heads lands in one [128, 2*CH]
                PSUM tile ([0:CH]=h0, [CH:2CH]=h1). Diagonal-band blocks run
                FIRST; their score/exp/PV work is restricted to the valid
                column range [128j, CH), and the triangle sub-block is masked
                on GPSIMD.

                `inject` is called after the diagonal group: the next chunk's
                projection matmuls slot into the PE stream there, overlapping
                the Activation engine's exp backlog so ACT never starves at
                chunk boundaries.

                `epi` is the PREVIOUS chunk's deferred epilogue; it is
                emitted just before this chunk's first PV so the reciprocal
                chain overlaps this chunk's diagonal scores instead of
                stalling the PE at the chunk boundary. Returns this chunk's
                epilogue closure for the same treatment."""
                kbmax = (qc + 1) * KBC
                psu = {}
                for h in (0, 1):
                    psu[h] = pspool.tile([65, CH], F32, tag="u",
                                         name=f"psu_{qc}_{h}")
                kb_order = (list(range(kbmax - KBC, kbmax))
                            + list(range(0, kbmax - KBC)))

                def emit_pv(kb, ki, pt, q0):
                    for h in (0, 1):
                        nc.tensor.matmul(
                            psu[h][:, q0:CH],
                            vnat[:, 130 * kb + 65 * h:
                                 130 * kb + 65 * (h + 1)],
                            pt[:, CH * h + q0:CH * (h + 1)],
                            start=(ki == 0),
                            stop=(ki == kbmax - 1),
                        )

                DEFER = 4       # PV trails exp by this many key blocks
                pending = []    # (kb, ki, pt, q0) with PV deferred
                inject = list(inject) if inject else []
                for ki, kb in enumerate(kb_order):
                    if ki == 1 and epi is not None:
                        epi()
                        epi = None
                    # dribble the next chunk's projection stages into the
                    # PE stream: DMAs right away, one compute stage per two
                    # key blocks after the diagonal group
                    if inject and (ki == 1 or
                                   (ki >= KBC and (ki - KBC) % 2 == 0)):
                        inject.pop(0)()
                    j = kb - (kbmax - KBC)  # diag index if >= 0
                    lastkb = (ki == kbmax - 1)
                    # column-restrict diagonal blocks except the last kb of
                    # the chunk (its stop=True must cover the full psu width)
                    q0 = 128 * j if (j >= 1 and not lastkb) else 0
                    pss = psspool.tile([P, 2 * CH], F32, tag="s",
                                       name=f"sc_{qc}_{kb}")
                    for h in (0, 1):
                        nc.tensor.matmul(
                            pss[:, CH * h + q0:CH * (h + 1)],
                            kT[64 * h:64 * (h + 1), bass.ts(kb, 128)],
                            qT[64 * h:64 * (h + 1),
                               qc * CH + q0:(qc + 1) * CH],
                            start=True,
                            stop=True,
                        )
                    pt = ptpool.tile([P, 2 * CH], BF16, tag="pt",
                                     name=f"pt_{qc}_{kb}")
                    if q0:
                        for h in (0, 1):
                            nc.scalar.activation(
                                pt[:, CH * h + q0:CH * (h + 1)],
                                pss[:, CH * h + q0:CH * (h + 1)],
                                EXP,
                            )
                    else:
                        nc.scalar.activation(pt[:], pss[:], EXP)
                    if j >= 0:
                        if lastkb and j >= 1:
                            # full-width last kb on a diagonal block (qc==0):
                            # zero the columns left of the band
                            for h in (0, 1):
                                nc.gpsimd.memset(
                                    pt[:, CH * h:CH * h + 128 * j], 0.0
                                )
                        for h in (0, 1):
                            o = CH * h + 128 * j
                            nc.gpsimd.tensor_mul(
                                pt[:, o:o + 128], pt[:, o:o + 128], tri[:]
                            )
                    if parts == "attn_s":
                        continue
                    pending.append((kb, ki, pt, q0))
                    if len(pending) > DEFER:
                        emit_pv(*pending.pop(0))
                while pending and parts != "attn_s":
                    emit_pv(*pending.pop(0))
                for f in inject:
                    f()
                if parts in ("attn_s", "attn_pv"):
                    return None

                def epilogue(final=False):
                    # normalize U by the denominator row and stage the bf16
                    # payload for the AllToAll; the final chunk's chain gates
                    # the AllToAll
                    rcp = []
                    with nc.allow_low_precision(
                            reason="f32r tag only; PE rounds on read"):
                        for h in (0, 1):
                            r = spool.tile([1, CH], F32R, tag=f"rs{h}",
                                           name=f"rs_{qc}_{h}")
                            nc.vector.reciprocal(r[:], psu[h][64:65, :])
                            rcp.append(r)
                    rb = pspool.tile([P, CH], F32, tag="mm", name=f"rb_{qc}")
                    nc.tensor.matmul(rb[:], e0[:], rcp[0][:],
                                     start=True, stop=False)
                    nc.tensor.matmul(rb[:], e1[:], rcp[1][:],
                                     start=False, stop=True)
                    # copy U halves into stg (copies may remap partitions),
                    # then normalize in place with partition-aligned muls
                    stg = spool.tile([P, CH], BF16, tag="stg",
                                     name=f"stg_{qc}")
                    # copy U halves into stg first: tensor-tensor ops may
                    # read at most one PSUM operand and need equal input
                    # base partitions, so the muls read stg(SB) x rb(PSUM).
                    # On the final chunk ACT is idle (exps done) and this
                    # chain gates the AllToAll: give it the h0 copy.
                    for h in (0, 1):
                        hs = slice(64 * h, 64 * (h + 1))
                        if final and h == 0:
                            nc.scalar.copy(stg[hs, :], psu[h][0:64, :])
                        else:
                            nc.vector.tensor_copy(stg[hs, :],
                                                  psu[h][0:64, :])
                        nc.vector.tensor_mul(stg[hs, :], stg[hs, :],
                                             rb[hs, :])
                    for half in (0, 1):
                        hs = bass.ts(half, CH // 2)
                        nc.sync.dma_start(out=a2a_in[qc, :, hs],
                                          in_=stg[:, hs])

                return epilogue

            def emit_p12():
                if parts.split("_")[0] in ("dma", "proj", "rope", "noil"):
                    for sc in range(NCH):
                        emit_proj_chunk(sc)
                    if parts == "noil":
                        for sc in range(NCH):
                            emit_attn_chunk(sc)
                    return
                emit_proj_chunk(0)
                for e in range(KD):
                    nc.sync.dma_start(
                        out=wo_sb[:, bass.ts(e, D_MODEL)],
                        in_=wo[:, bass.ts(e, D_MODEL)],
                    )
                DEFER_EPI = True
                epi = None
                for qc in range(NCH):
                    nxt = qc + 1
                    ret = emit_attn_chunk(
                        qc,
                        inject=proj_stages(nxt) if nxt < NCH else None,
                        epi=epi,
                    )
                    if DEFER_EPI:
                        epi = ret
                    elif ret is not None:
                        ret()
                if epi is not None:
                    epi(final=True)

            def emit_cc():
                nc.gpsimd.collective_compute(
                    "AllToAll",
                    mybir.AluOpType.bypass,
                    replica_groups=[list(range(N_CORES))],
                    ins=[a2a_in.opt()],
                    outs=[a2a_out.opt()],
                )

            def emit_p3(cc_done):
                if not cc_done:
                    emit_cc()
                ats = []
                for i in range(N_CORES):
                    at = apool.tile([P, SW], BF16, tag=f"at{i}",
                                    name=f"at_{i}")
                    eng = nc.sync if i % 2 == 0 else nc.scalar
                    eng.dma_start(out=at[:], in_=a2a_out[i])
                    ats.append(at)
                for e in range(KD):
                    pso = pspool.tile([P, SW], F32, tag="mm",
                                      name=f"pso_{e}")
                    for i in range(N_CORES):
                        nc.tensor.matmul(
                            pso[:],
                            wo_sb[:, D_MODEL * e + 128 * i:
                                  D_MODEL * e + 128 * (i + 1)],
                            ats[i][:],
                            start=(i == 0),
                            stop=(i == N_CORES - 1),
                        )
                    ot = ptpool.tile([P, SW], BF16, tag="ot", name=f"ot_{e}")
                    # alternate the PSUM drain between DVE and ACT (both
                    # idle in phase 3) and spread out DMAs over two rings
                    if e % 2 == 0:
                        nc.vector.tensor_copy(ot[:], pso[:])
                    else:
                        nc.scalar.copy(ot[:], pso[:])
                    eng = nc.sync if e % 2 == 0 else nc.scalar
                    eng.dma_start(out=out_d[bass.ts(e, 128)], in_=ot[:])

            if p12_reps == 1:
                emit_p12()
                if parts == "full":
                    emit_cc()
            else:
                with tc.For_i(0, p12_reps, 1):
                    emit_p12()
            if parts == "full":
                for r3 in range(p3_reps):
                    emit_p3(cc_done=(p12_reps == 1 and r3 == 0))

    nc.finalize()
    return nc


def prepare_in_maps(in_features, token_positions, Wq, Wk, Wv, Wo, seq):
    """Host-side staging: shard/transform full inputs into per-core maps."""
    import ml_dtypes
    bf16 = ml_dtypes.bfloat16
    x = np.ascontiguousarray(np.asarray(in_features, dtype=np.float32)[0])
    pos = np.asarray(token_positions).reshape(-1)[:seq].astype(np.float64)

    xt = np.ascontiguousarray(x.T).astype(bf16)  # [D, S]

    # RoPE tables in rotate-half form after pair permutation.
    inv_freq = THETA ** (-np.arange(0, HEAD_DIM, 2, dtype=np.float64)
                         / HEAD_DIM)
    ang = pos[:, None] * inv_freq[None, :]  # [S, 32]
    cos = np.cos(ang).T.astype(np.float32)  # [32, S]
    sin = np.sin(ang).T.astype(np.float32)
    ctab = np.ascontiguousarray(np.tile(cos, (4, 1))).astype(bf16)
    stab = np.ascontiguousarray(
        np.concatenate([-sin, sin, -sin, sin], axis=0)
    ).astype(bf16)

    perm = np.concatenate(
        [np.arange(0, HEAD_DIM, 2), np.arange(1, HEAD_DIM, 2)]
    )  # within-head: evens then odds

    tri = np.triu(np.ones((128, 128), dtype=np.float32)).astype(bf16)
    e0_host = np.zeros((1, 128), dtype=np.float32)
    e0_host[0, 0:64] = 1.0
    e1_host = np.zeros((1, 128), dtype=np.float32)
    e1_host[0, 64:128] = 1.0

    import ml_dtypes as _mld
    WoT = np.ascontiguousarray(np.asarray(Wo, dtype=np.float32).T)  # [d, e]
    wo_packed = np.empty((128, KD * D_MODEL), dtype=np.float32)
    for e in range(KD):
        for i in range(KD):
            wo_packed[:, D_MODEL * e + 128 * i: D_MODEL * e + 128 * (i + 1)] \
                = WoT[128 * i:128 * (i + 1), 128 * e:128 * (e + 1)]
    wo_packed = wo_packed.astype(bf16)

    def pack_w(Wc):
        # Wc: [128 out, 1024 in] -> WT [1024, 128] -> [128, 8*128] k-tiled
        WT = np.ascontiguousarray(Wc.T)
        return np.ascontiguousarray(
            WT.reshape(KD, 128, 128).transpose(1, 0, 2).reshape(128, KD * 128)
        ).astype(bf16)

    in_maps = []
    for c in range(N_CORES):
        rows = slice(128 * c, 128 * (c + 1))
        Wq_r = np.asarray(Wq, dtype=np.float32)[rows].reshape(2, 64, D_MODEL)
        Wq_c = (Wq_r[:, perm, :] / math.sqrt(HEAD_DIM)).reshape(128, D_MODEL)
        Wk_r = np.asarray(Wk, dtype=np.float32)[rows].reshape(2, 64, D_MODEL)
        Wk_c = Wk_r[:, perm, :].reshape(128, D_MODEL)
        # wvt block j = WvT[128j:128(j+1), :] = Wv_core[:, 128j:...].T
        WvT = np.ascontiguousarray(
            np.asarray(Wv, dtype=np.float32)[rows].T
        )  # [1024 in, 128 out]
        wvt_packed = np.ascontiguousarray(
            WvT.reshape(KD, 128, 128).transpose(1, 0, 2).reshape(
                128, KD * 128)
        ).astype(bf16)
        in_maps.append({
            "xt": xt,
            "wq": pack_w(Wq_c),
            "wk": pack_w(Wk_c),
            "wvt": wvt_packed,
            "wo": wo_packed,
            "ctab": ctab,
            "stab": stab,
            "tri": tri,
            "e0": e0_host,
            "e1": e1_host,
        })
    return in_maps


_BUILD_CACHE = {}


def _get_nc(seq, p12_reps=1, p3_reps=1, parts="full"):
    key = (seq, p12_reps, p3_reps, parts)
    if key not in _BUILD_CACHE:
        _BUILD_CACHE[key] = build(seq, p12_reps, p3_reps, parts)
    return _BUILD_CACHE[key]


def postprocess(results, seq, in_dtype):
    SW = seq // N_CORES
    out = np.empty((seq, D_MODEL), dtype=np.float32)
    for c in range(N_CORES):
        out[SW * c:SW * (c + 1), :] = results[c]["out"].T
    return out.reshape(1, seq, D_MODEL).astype(in_dtype)


def kernel(in_features, token_positions, Wq, Wk, Wv, Wo):
    in_dtype = np.asarray(in_features).dtype
    B, S, D = np.asarray(in_features).shape
    assert B == 1 and D == D_MODEL

    nc = _get_nc(S)
    in_maps = prepare_in_maps(in_features, token_positions, Wq, Wk, Wv, Wo, S)
    res = run_bass_kernel_spmd(nc, in_maps, list(range(N_CORES)), trace=False)
    return postprocess(res.results, S, in_dtype)
